# revision 2
# baseline (speedup 1.0000x reference)
"""Trainium2 Bass kernel for nn_DotAtt_40097814675537.

Math (matches the reference exactly up to fp rounding):
    score = Q @ K^T / sqrt(d)        [B, Sq, Sk]
    x     = score @ V                [B, Sq, dv]
    out   = softmax(where(j > valid_len[q], -1e6, x[b, q, j]), axis=-1)

Since there is no nonlinearity between the two matmuls, we use associativity:
    x = (Q / sqrt(d)) @ (K^T @ V)
which cuts FLOPs 4x (contraction 2048 -> 512 for the big matmul).

Sharding: data-parallel over batch B=8, one batch per NeuronCore (8 cores).
valid_len is replicated. Each core computes:
    M  = K_b^T @ V_b               (512 x 512, contraction 2048)
    X  = (Q_b / sqrt(d)) @ M       (2048 x 512, contraction 512)
    out_b = softmax(X + maskadd)   row-wise over 512, maskadd = -1e6 * (j > vl[q])

Host-side prep (cheap numpy): per-batch shard, Q transpose+scale (so the
stationary matmul operand needs no on-device transpose), valid_len as an
fp32 [128, 16] tile.
"""

import math
import sys
import types

import numpy as np

B, SQ, SK, D, DV = 8, 2048, 2048, 512, 512
N_CORES = 8
P = 128  # partitions
SC = SK // P  # 16 s-chunks for the K^T V contraction
DC = D // P  # 4 d-chunks for the Q M contraction
QT_TILES = SQ // P  # 16 query row tiles
NEG_FILL = -1000000.0

# Matmul input dtype mode: "fp32" (safe), "f32r" (fast, reduced precision)
MM_MODE = "fp32"

_CACHE = {}


def _install_ntff_hook():
    """antenv.axon_hooks is absent in this image; provide it so trace=True
    profiling works when requested (used by test.py, harmless otherwise)."""
    if "antenv.axon_hooks" in sys.modules:
        return
    try:
        from trn_agent_boot.trn_boot import _ntff_profile_via_ctypes

        hook = _ntff_profile_via_ctypes("/opt/axon/libaxon_pjrt.so")
    except Exception:
        hook = None
    mod = types.ModuleType("antenv.axon_hooks")
    mod.get_axon_ntff_profile_hook = lambda: hook
    mod.set_axon_ntff_profile_hook = lambda h: None
    sys.modules["antenv.axon_hooks"] = mod


def _build(mm_mode: str):
    import concourse.tile as tile
    from concourse import bacc, mybir

    nc = bacc.Bacc("TRN2", target_bir_lowering=False, debug=False, num_devices=N_CORES)
    f32 = mybir.dt.float32
    mm_dt = f32 if mm_mode == "fp32" else mybir.dt.float32r

    k_d = nc.dram_tensor("k", [SK, D], f32, kind="ExternalInput")
    v_d = nc.dram_tensor("v", [SK, DV], f32, kind="ExternalInput")
    qt_d = nc.dram_tensor("qt", [D, SQ], f32, kind="ExternalInput")  # pre-scaled Q^T
    vl_d = nc.dram_tensor("vl", [P, QT_TILES], f32, kind="ExternalInput")
    o_d = nc.dram_tensor("o", [SQ, DV], f32, kind="ExternalOutput")

    def mm(ap):
        return ap.bitcast(mm_dt) if mm_mode != "fp32" else ap

    with tile.TileContext(nc) as tc:
        with (
            tc.tile_pool(name="consts", bufs=1) as consts,
            tc.tile_pool(name="qt", bufs=1) as qt_pool,
            tc.tile_pool(name="kv", bufs=3) as kv_pool,
            tc.tile_pool(name="mprime", bufs=1) as mp_pool,
            tc.tile_pool(name="psm", bufs=1, space="PSUM") as psum_m,
            tc.tile_pool(name="psx", bufs=3, space="PSUM") as psum_x,
            tc.tile_pool(name="work", bufs=3) as work,
            tc.tile_pool(name="stats", bufs=4) as stats,
        ):
            # constants: valid_len tile and the column-index iota
            vl_t = consts.tile([P, QT_TILES], f32, tag="vl")
            nc.sync.dma_start(out=vl_t, in_=vl_d[:, :])
            iota_t = consts.tile([P, DV], f32, tag="iota")
            nc.gpsimd.iota(
                iota_t,
                pattern=[[1, DV]],
                base=0,
                channel_multiplier=0,
                allow_small_or_imprecise_dtypes=True,
            )

            # resident Q^T (pre-scaled by 1/sqrt(d) on host): 4 chunks [128, 2048]
            qts = []
            for c in range(DC):
                t = qt_pool.tile([P, SQ], f32, tag=f"qt{c}", name=f"qt_sb{c}")
                nc.sync.dma_start(out=t, in_=qt_d[c * P : (c + 1) * P, :])
                qts.append(t)

            # Phase 1: M = K^T V, accumulated over 16 s-chunks into 4 PSUM banks
            psums = [psum_m.tile([P, DV], f32, tag=f"m{c}", name=f"psum_m{c}") for c in range(DC)]
            for s in range(SC):
                kt = kv_pool.tile([P, D], f32, tag="k")
                nc.sync.dma_start(out=kt, in_=k_d[s * P : (s + 1) * P, :])
                vt = kv_pool.tile([P, DV], f32, tag="v")
                nc.sync.dma_start(out=vt, in_=v_d[s * P : (s + 1) * P, :])
                for c in range(DC):
                    nc.tensor.matmul(
                        psums[c][:, :],
                        mm(kt[:, c * P : (c + 1) * P]),
                        mm(vt[:, :]),
                        start=(s == 0),
                        stop=(s == SC - 1),
                    )

            # M PSUM -> SBUF (ScalarE copies; PE and DVE stay free)
            ms = []
            for c in range(DC):
                m_sb = mp_pool.tile([P, DV], f32, tag=f"ms{c}", name=f"m_sb{c}")
                nc.scalar.copy(m_sb[:, :], psums[c][:, :])
                ms.append(m_sb)

            # Phase 2: per query tile: X = Q M, mask, softmax, store
            for t in range(QT_TILES):
                px = psum_x.tile([P, DV], f32, tag="x")
                for c in range(DC):
                    nc.tensor.matmul(
                        px[:, :],
                        mm(qts[c][:, t * P : (t + 1) * P]),
                        mm(ms[c][:, :]),
                        start=(c == 0),
                        stop=(c == DC - 1),
                    )
                # additive mask on GpSimd: (iota > vl) * -1e6
                maskt = work.tile([P, DV], f32, tag="mask")
                nc.gpsimd.tensor_scalar(
                    out=maskt,
                    in0=iota_t,
                    scalar1=vl_t[:, t : t + 1],
                    scalar2=NEG_FILL,
                    op0=mybir.AluOpType.is_gt,
                    op1=mybir.AluOpType.mult,
                )
                xs = work.tile([P, DV], f32, tag="x")
                nc.vector.tensor_add(xs[:, :], px[:, :], maskt[:, :])
                nmx = stats.tile([P, 1], f32, tag="nmx")
                nc.vector.tensor_reduce(
                    out=nmx,
                    in_=xs[:, :],
                    axis=mybir.AxisListType.X,
                    op=mybir.AluOpType.max,
                    negate=True,
                )
                ex = work.tile([P, DV], f32, tag="e")
                sm = stats.tile([P, 1], f32, tag="sum")
                nc.scalar.activation(
                    ex[:, :],
                    xs[:, :],
                    mybir.ActivationFunctionType.Exp,
                    bias=nmx[:, :],
                    scale=1.0,
                    accum_out=sm[:, :],
                )
                rs = stats.tile([P, 1], f32, tag="r")
                nc.vector.reciprocal(rs, sm)
                ot = work.tile([P, DV], f32, tag="o")
                nc.vector.tensor_scalar_mul(ot[:, :], ex[:, :], rs[:, :])
                nc.sync.dma_start(out=o_d[t * P : (t + 1) * P, :], in_=ot[:, :])

    nc.compile()
    return nc


def _get_nc(mm_mode: str):
    if mm_mode not in _CACHE:
        _install_ntff_hook()
        _CACHE[mm_mode] = _build(mm_mode)
    return _CACHE[mm_mode]


def kernel(K, V, Q, valid_len, _trace=False, _mm_mode=None):
    mm_mode = _mm_mode or MM_MODE
    from concourse.bass_utils import run_bass_kernel_spmd

    K = np.ascontiguousarray(np.asarray(K, dtype=np.float32))
    V = np.ascontiguousarray(np.asarray(V, dtype=np.float32))
    Q = np.asarray(Q, dtype=np.float32)
    vl = np.asarray(valid_len)

    # Q^T per batch, pre-scaled by 1/sqrt(d)
    scale = np.float32(1.0 / math.sqrt(D))
    qt = np.ascontiguousarray((Q * scale).transpose(0, 2, 1))
    # valid_len as fp32 [128, 16] tile: vl_t[p, t] = valid_len[t*128 + p]
    vl_t = np.ascontiguousarray(
        vl.astype(np.float32).reshape(QT_TILES, P).T
    )

    nc = _get_nc(mm_mode)
    in_maps = [
        {"k": K[b], "v": V[b], "qt": qt[b], "vl": vl_t} for b in range(N_CORES)
    ]
    res = run_bass_kernel_spmd(
        nc, in_maps, core_ids=list(range(N_CORES)), trace=_trace
    )
    out = np.stack([res.results[b]["o"] for b in range(N_CORES)], axis=0)
    if _trace:
        kernel.last_result = res
    return out


# revision 7
# speedup vs baseline: 1.5923x; 1.5923x over previous
"""Trainium2 Bass kernel for nn_DotAtt_40097814675537.

Math (matches the reference exactly up to fp rounding):
    score = Q @ K^T / sqrt(d)        [B, Sq, Sk]
    x     = score @ V                [B, Sq, dv]
    out   = softmax(where(j > valid_len[q], -1e6, x[b, q, j]), axis=-1)

Since there is no nonlinearity between the two matmuls, we use associativity:
    x = (Q / sqrt(d)) @ (K^T @ V)
which cuts FLOPs 4x (contraction 2048 -> 512 for the big matmul).

Sharding: data-parallel over batch B=8, one batch per NeuronCore (8 cores).
valid_len is replicated. Each core computes:
    M  = K_b^T @ V_b               (512 x 512, contraction 2048)
    X  = (Q_b / sqrt(d)) @ M       (2048 x 512, contraction 512)
    out_b = softmax(X + maskadd)   row-wise over 512, maskadd = -1e6 * (j > vl[q])

Host-side prep (cheap numpy): per-batch shard, Q transpose+scale (so the
stationary matmul operand needs no on-device transpose), valid_len as an
fp32 [128, 16] tile.
"""

import math
import sys
import types

import numpy as np

B, SQ, SK, D, DV = 8, 2048, 2048, 512, 512
N_CORES = 8
P = 128  # partitions
SC = SK // P  # 16 s-chunks for the K^T V contraction
DC = D // P  # 4 d-chunks for the Q M contraction
QT_TILES = SQ // P  # 16 query row tiles
NEG_FILL = -1000000.0

# Matmul input dtype mode: "fp32" (safe), "f32r" (fast, reduced precision)
MM_MODE = "fp32"

_CACHE = {}


def _install_ntff_hook():
    """antenv.axon_hooks is absent in this image; provide it so trace=True
    profiling works when requested (used by test.py, harmless otherwise)."""
    if "antenv.axon_hooks" in sys.modules:
        return
    try:
        from trn_agent_boot.trn_boot import _ntff_profile_via_ctypes

        hook = _ntff_profile_via_ctypes("/opt/axon/libaxon_pjrt.so")
    except Exception:
        hook = None
    mod = types.ModuleType("antenv.axon_hooks")
    mod.get_axon_ntff_profile_hook = lambda: hook
    mod.set_axon_ntff_profile_hook = lambda h: None
    sys.modules["antenv.axon_hooks"] = mod


def _build(mm_mode: str):
    import concourse.tile as tile
    from concourse import bacc, mybir

    nc = bacc.Bacc("TRN2", target_bir_lowering=False, debug=False, num_devices=N_CORES)
    f32 = mybir.dt.float32
    mm_dt = f32 if mm_mode == "fp32" else mybir.dt.float32r

    bf16 = mybir.dt.bfloat16
    k_d = nc.dram_tensor("k", [SK, D], f32, kind="ExternalInput")
    v_d = nc.dram_tensor("v", [SK, DV], f32, kind="ExternalInput")
    qt_d = nc.dram_tensor("qt", [D, SQ], f32, kind="ExternalInput")  # pre-scaled Q^T
    mask_d = nc.dram_tensor("mask", [SQ, DV], bf16, kind="ExternalInput")
    o_d = nc.dram_tensor("o", [SQ, DV], f32, kind="ExternalOutput")

    def mm(ap):
        return ap.bitcast(mm_dt) if mm_mode != "fp32" else ap

    with tile.TileContext(nc) as tc:
        with (
            tc.tile_pool(name="consts", bufs=1) as consts,
            tc.tile_pool(name="qt", bufs=1) as qt_pool,
            tc.tile_pool(name="kv", bufs=3) as kv_pool,
            tc.tile_pool(name="mprime", bufs=1) as mp_pool,
            tc.tile_pool(name="psm", bufs=1, space="PSUM") as psum_m,
            tc.tile_pool(name="psx", bufs=3, space="PSUM") as psum_x,
            tc.tile_pool(name="work", bufs=3) as work,
            tc.tile_pool(name="stats", bufs=4) as stats,
        ):
            # resident Q^T (pre-scaled by 1/sqrt(d) on host): 4 chunks [128, 2048]
            qts = []
            for c in range(DC):
                t = qt_pool.tile([P, SQ], f32, tag=f"qt{c}", name=f"qt_sb{c}")
                nc.sync.dma_start(out=t, in_=qt_d[c * P : (c + 1) * P, :])
                qts.append(t)

            # Phase 1: M = K^T V, accumulated over 16 s-chunks into 4 PSUM banks
            psums = [psum_m.tile([P, DV], f32, tag=f"m{c}", name=f"psum_m{c}") for c in range(DC)]
            for s in range(SC):
                kt = kv_pool.tile([P, D], f32, tag="k")
                nc.sync.dma_start(out=kt, in_=k_d[s * P : (s + 1) * P, :])
                vt = kv_pool.tile([P, DV], f32, tag="v")
                nc.sync.dma_start(out=vt, in_=v_d[s * P : (s + 1) * P, :])
                for c in range(DC):
                    nc.tensor.matmul(
                        psums[c][:, :],
                        mm(kt[:, c * P : (c + 1) * P]),
                        mm(vt[:, :]),
                        start=(s == 0),
                        stop=(s == SC - 1),
                    )

            # M PSUM -> SBUF (ScalarE copies; PE and DVE stay free)
            ms = []
            for c in range(DC):
                m_sb = mp_pool.tile([P, DV], f32, tag=f"ms{c}", name=f"m_sb{c}")
                nc.scalar.copy(m_sb[:, :], psums[c][:, :])
                ms.append(m_sb)

            # Phase 2: per query tile: X = Q M, mask, softmax, store
            for t in range(QT_TILES):
                px = psum_x.tile([P, DV], f32, tag="x")
                for c in range(DC):
                    nc.tensor.matmul(
                        px[:, :],
                        mm(qts[c][:, t * P : (t + 1) * P]),
                        mm(ms[c][:, :]),
                        start=(c == 0),
                        stop=(c == DC - 1),
                    )
                # additive mask (host-precomputed, bf16; exact: only 0 / -1e6-ish)
                maskt = work.tile([P, DV], bf16, tag="mask")
                nc.sync.dma_start(out=maskt, in_=mask_d[t * P : (t + 1) * P, :])
                xs = work.tile([P, DV], f32, tag="x")
                nc.vector.tensor_add(xs[:, :], px[:, :], maskt[:, :])
                nmx = stats.tile([P, 1], f32, tag="nmx")
                nc.vector.tensor_reduce(
                    out=nmx,
                    in_=xs[:, :],
                    axis=mybir.AxisListType.X,
                    op=mybir.AluOpType.max,
                    negate=True,
                )
                ex = work.tile([P, DV], f32, tag="e")
                sm = stats.tile([P, 1], f32, tag="sum")
                nc.scalar.activation(
                    ex[:, :],
                    xs[:, :],
                    mybir.ActivationFunctionType.Exp,
                    bias=nmx[:, :],
                    scale=1.0,
                    accum_out=sm[:, :],
                )
                rs = stats.tile([P, 1], f32, tag="r")
                nc.vector.reciprocal(rs, sm)
                ot = work.tile([P, DV], f32, tag="o")
                # normalize on ScalarE (ACT copy with per-partition scale)
                nc.scalar.mul(ot[:, :], ex[:, :], rs[:, :])
                nc.sync.dma_start(out=o_d[t * P : (t + 1) * P, :], in_=ot[:, :])

    nc.compile()
    return nc


def _get_nc(mm_mode: str):
    if mm_mode not in _CACHE:
        _install_ntff_hook()
        _CACHE[mm_mode] = _build(mm_mode)
    return _CACHE[mm_mode]


def kernel(K, V, Q, valid_len, _trace=False, _mm_mode=None):
    mm_mode = _mm_mode or MM_MODE
    from concourse.bass_utils import run_bass_kernel_spmd

    K = np.ascontiguousarray(np.asarray(K, dtype=np.float32))
    V = np.ascontiguousarray(np.asarray(V, dtype=np.float32))
    Q = np.asarray(Q, dtype=np.float32)
    vl = np.asarray(valid_len)

    # Q^T per batch, pre-scaled by 1/sqrt(d)
    scale = np.float32(1.0 / math.sqrt(D))
    qt = np.ascontiguousarray((Q * scale).transpose(0, 2, 1))
    # additive mask [Sq, dv] in bf16 (values are exactly 0 or ~-1e6; masked
    # lanes underflow to 0 after exp either way, identical to masked_fill)
    import ml_dtypes

    maskadd = np.where(
        np.arange(DV, dtype=np.int64)[None, :] > vl.astype(np.int64)[:, None],
        np.float32(NEG_FILL),
        np.float32(0.0),
    ).astype(ml_dtypes.bfloat16)

    nc = _get_nc(mm_mode)
    in_maps = [
        {"k": K[b], "v": V[b], "qt": qt[b], "mask": maskadd} for b in range(N_CORES)
    ]
    res = run_bass_kernel_spmd(
        nc, in_maps, core_ids=list(range(N_CORES)), trace=_trace
    )
    out = np.stack([res.results[b]["o"] for b in range(N_CORES)], axis=0)
    if _trace:
        kernel.last_result = res
    return out


# revision 11
# speedup vs baseline: 1.8666x; 1.1722x over previous
"""Trainium2 Bass kernel for nn_DotAtt_40097814675537.

Math (matches the reference exactly up to fp rounding):
    score = Q @ K^T / sqrt(d)        [B, Sq, Sk]
    x     = score @ V                [B, Sq, dv]
    out   = softmax(where(j > valid_len[q], -1e6, x[b, q, j]), axis=-1)

Since there is no nonlinearity between the two matmuls, we use associativity:
    x = (Q / sqrt(d)) @ (K^T @ V)
which cuts FLOPs 4x (contraction 2048 -> 512 for the big matmul).

Sharding: data-parallel over batch B=8, one batch per NeuronCore (8 cores).
valid_len is replicated. Each core computes:
    M  = K_b^T @ V_b               (512 x 512, contraction 2048)
    X  = (Q_b / sqrt(d)) @ M       (2048 x 512, contraction 512)
    out_b = softmax(X + maskadd)   row-wise over 512, maskadd = -1e6 * (j > vl[q])

Host-side prep (cheap numpy): per-batch shard, Q transpose+scale (so the
stationary matmul operand needs no on-device transpose), valid_len as an
fp32 [128, 16] tile.
"""

import math
import sys
import types

import numpy as np

B, SQ, SK, D, DV = 8, 2048, 2048, 512, 512
N_CORES = 8
P = 128  # partitions
SC = SK // P  # 16 s-chunks for the K^T V contraction
DC = D // P  # 4 d-chunks for the Q M contraction
QT_TILES = SQ // P  # 16 query row tiles
NEG_FILL = -1000000.0

# Matmul input dtype mode:
#   "fp32"   - native fp32 matmuls (4 cyc/row, safest)
#   "fp16x3" - hi/lo float16 split, 3 passes (3 cyc/row, fp32-class accuracy)
MM_MODE = "fp16x3"

_CACHE = {}


def _install_ntff_hook():
    """antenv.axon_hooks is absent in this image; provide it so trace=True
    profiling works when requested (used by test.py, harmless otherwise)."""
    if "antenv.axon_hooks" in sys.modules:
        return
    try:
        from trn_agent_boot.trn_boot import _ntff_profile_via_ctypes

        hook = _ntff_profile_via_ctypes("/opt/axon/libaxon_pjrt.so")
    except Exception:
        hook = None
    mod = types.ModuleType("antenv.axon_hooks")
    mod.get_axon_ntff_profile_hook = lambda: hook
    mod.set_axon_ntff_profile_hook = lambda h: None
    sys.modules["antenv.axon_hooks"] = mod


def _build(mm_mode: str):
    if mm_mode == "fp16x3":
        return _build_fp16x3()
    import concourse.tile as tile
    from concourse import bacc, mybir

    nc = bacc.Bacc("TRN2", target_bir_lowering=False, debug=False, num_devices=N_CORES)
    f32 = mybir.dt.float32
    mm_dt = f32 if mm_mode == "fp32" else mybir.dt.float32r

    bf16 = mybir.dt.bfloat16
    k_d = nc.dram_tensor("k", [SK, D], f32, kind="ExternalInput")
    v_d = nc.dram_tensor("v", [SK, DV], f32, kind="ExternalInput")
    qt_d = nc.dram_tensor("qt", [D, SQ], f32, kind="ExternalInput")  # pre-scaled Q^T
    mask_d = nc.dram_tensor("mask", [SQ, DV], bf16, kind="ExternalInput")
    o_d = nc.dram_tensor("o", [SQ, DV], f32, kind="ExternalOutput")

    def mm(ap):
        return ap.bitcast(mm_dt) if mm_mode != "fp32" else ap

    with tile.TileContext(nc) as tc:
        with (
            tc.tile_pool(name="consts", bufs=1) as consts,
            tc.tile_pool(name="qt", bufs=1) as qt_pool,
            tc.tile_pool(name="kv", bufs=3) as kv_pool,
            tc.tile_pool(name="mprime", bufs=1) as mp_pool,
            tc.tile_pool(name="psm", bufs=1, space="PSUM") as psum_m,
            tc.tile_pool(name="psx", bufs=3, space="PSUM") as psum_x,
            tc.tile_pool(name="work", bufs=3) as work,
            tc.tile_pool(name="stats", bufs=4) as stats,
        ):
            # resident Q^T (pre-scaled by 1/sqrt(d) on host): 4 chunks [128, 2048]
            qts = []
            for c in range(DC):
                t = qt_pool.tile([P, SQ], f32, tag=f"qt{c}", name=f"qt_sb{c}")
                nc.sync.dma_start(out=t, in_=qt_d[c * P : (c + 1) * P, :])
                qts.append(t)

            # Phase 1: M = K^T V, accumulated over 16 s-chunks into 4 PSUM banks
            psums = [psum_m.tile([P, DV], f32, tag=f"m{c}", name=f"psum_m{c}") for c in range(DC)]
            for s in range(SC):
                kt = kv_pool.tile([P, D], f32, tag="k")
                nc.sync.dma_start(out=kt, in_=k_d[s * P : (s + 1) * P, :])
                vt = kv_pool.tile([P, DV], f32, tag="v")
                nc.sync.dma_start(out=vt, in_=v_d[s * P : (s + 1) * P, :])
                for c in range(DC):
                    nc.tensor.matmul(
                        psums[c][:, :],
                        mm(kt[:, c * P : (c + 1) * P]),
                        mm(vt[:, :]),
                        start=(s == 0),
                        stop=(s == SC - 1),
                    )

            # M PSUM -> SBUF (ScalarE copies; PE and DVE stay free)
            ms = []
            for c in range(DC):
                m_sb = mp_pool.tile([P, DV], f32, tag=f"ms{c}", name=f"m_sb{c}")
                nc.scalar.copy(m_sb[:, :], psums[c][:, :])
                ms.append(m_sb)

            # Phase 2: per query tile: X = Q M, mask, softmax, store
            for t in range(QT_TILES):
                px = psum_x.tile([P, DV], f32, tag="x")
                for c in range(DC):
                    nc.tensor.matmul(
                        px[:, :],
                        mm(qts[c][:, t * P : (t + 1) * P]),
                        mm(ms[c][:, :]),
                        start=(c == 0),
                        stop=(c == DC - 1),
                    )
                # additive mask (host-precomputed, bf16; exact: only 0 / -1e6-ish)
                maskt = work.tile([P, DV], bf16, tag="mask")
                nc.sync.dma_start(out=maskt, in_=mask_d[t * P : (t + 1) * P, :])
                xs = work.tile([P, DV], f32, tag="x")
                nc.vector.tensor_add(xs[:, :], px[:, :], maskt[:, :])
                nmx = stats.tile([P, 1], f32, tag="nmx")
                nc.vector.tensor_reduce(
                    out=nmx,
                    in_=xs[:, :],
                    axis=mybir.AxisListType.X,
                    op=mybir.AluOpType.max,
                    negate=True,
                )
                ex = work.tile([P, DV], f32, tag="e")
                sm = stats.tile([P, 1], f32, tag="sum")
                nc.scalar.activation(
                    ex[:, :],
                    xs[:, :],
                    mybir.ActivationFunctionType.Exp,
                    bias=nmx[:, :],
                    scale=1.0,
                    accum_out=sm[:, :],
                )
                rs = stats.tile([P, 1], f32, tag="r")
                nc.vector.reciprocal(rs, sm)
                ot = work.tile([P, DV], f32, tag="o")
                # normalize on ScalarE (ACT copy with per-partition scale)
                nc.scalar.mul(ot[:, :], ex[:, :], rs[:, :])
                nc.sync.dma_start(out=o_d[t * P : (t + 1) * P, :], in_=ot[:, :])

    nc.compile()
    return nc


def _build_fp16x3():
    """fp32-accurate matmuls from 3 float16 passes.

    Each fp32 operand x is split (on host for K/V/Q, on device for M) into
    x = hi + lo with hi = fp16(x), lo = fp16(x - hi).  Then
        a @ b ~= ah@bh + ah@bl + al@bh     (al@bl ~ 2^-22 rel, dropped)
    Every pass runs at the fp16 PE rate (1 cyc/row vs 4 for native fp32).
    Inputs arrive packed along the free dim: [rows, 2*cols] = [hi | lo].
    """
    import concourse.tile as tile
    from concourse import bacc, mybir

    nc = bacc.Bacc("TRN2", target_bir_lowering=False, debug=False, num_devices=N_CORES)
    f32 = mybir.dt.float32
    f16 = mybir.dt.float16
    bf16 = mybir.dt.bfloat16

    k_d = nc.dram_tensor("k", [SK, 2 * D], f16, kind="ExternalInput")  # [hi|lo]
    v_d = nc.dram_tensor("v", [SK, 2 * DV], f16, kind="ExternalInput")
    qt_d = nc.dram_tensor("qt", [D, 2 * SQ], f16, kind="ExternalInput")  # scaled
    mask_d = nc.dram_tensor("mask", [SQ, DV], bf16, kind="ExternalInput")
    o_d = nc.dram_tensor("o", [SQ, DV], f32, kind="ExternalOutput")

    with tile.TileContext(nc) as tc:
        with (
            tc.tile_pool(name="qt", bufs=1) as qt_pool,
            tc.tile_pool(name="kv", bufs=3) as kv_pool,
            tc.tile_pool(name="mprime", bufs=1) as mp_pool,
            tc.tile_pool(name="psm", bufs=1, space="PSUM") as psum_m,
            tc.tile_pool(name="psx", bufs=3, space="PSUM") as psum_x,
            tc.tile_pool(name="work", bufs=3) as work,
            tc.tile_pool(name="stats", bufs=4) as stats,
        ):
            # resident packed Q^T: 4 chunks [128, 2*2048]
            qts = []
            for c in range(DC):
                t = qt_pool.tile([P, 2 * SQ], f16, tag=f"qt{c}", name=f"qt_sb{c}")
                nc.sync.dma_start(out=t, in_=qt_d[c * P : (c + 1) * P, :])
                qts.append(t)

            # Phase 1: M = K^T V over 16 s-chunks, 3 passes each
            psums = [
                psum_m.tile([P, DV], f32, tag=f"m{c}", name=f"psum_m{c}")
                for c in range(DC)
            ]
            for s in range(SC):
                kt = kv_pool.tile([P, 2 * D], f16, tag="k")
                nc.sync.dma_start(out=kt, in_=k_d[s * P : (s + 1) * P, :])
                vt = kv_pool.tile([P, 2 * DV], f16, tag="v")
                nc.sync.dma_start(out=vt, in_=v_d[s * P : (s + 1) * P, :])
                vh = vt[:, 0:DV]
                vlo = vt[:, DV : 2 * DV]
                for c in range(DC):
                    kh = kt[:, c * P : (c + 1) * P]
                    klo = kt[:, D + c * P : D + (c + 1) * P]
                    # same-weight passes adjacent to reuse the loaded weights
                    nc.tensor.matmul(
                        psums[c][:, :], kh, vh, start=(s == 0), stop=False
                    )
                    nc.tensor.matmul(psums[c][:, :], kh, vlo, start=False, stop=False)
                    nc.tensor.matmul(
                        psums[c][:, :], klo, vh, start=False, stop=(s == SC - 1)
                    )

            # M PSUM -> SBUF split into fp16 hi/lo (ScalarE cast + DVE residual)
            mhis, mlos = [], []
            for c in range(DC):
                mhi = mp_pool.tile([P, DV], f16, tag=f"mh{c}", name=f"mhi{c}")
                nc.scalar.copy(mhi[:, :], psums[c][:, :])
                mlo = mp_pool.tile([P, DV], f16, tag=f"ml{c}", name=f"mlo{c}")
                nc.vector.tensor_sub(mlo[:, :], psums[c][:, :], mhi[:, :])
                mhis.append(mhi)
                mlos.append(mlo)

            # Phase 2: per query tile: X = Q M (12 passes), mask, softmax, store
            for t in range(QT_TILES):
                px = psum_x.tile([P, DV], f32, tag="x")
                for c in range(DC):
                    qh = qts[c][:, t * P : (t + 1) * P]
                    qlo = qts[c][:, SQ + t * P : SQ + (t + 1) * P]
                    nc.tensor.matmul(
                        px[:, :], qh, mhis[c][:, :], start=(c == 0), stop=False
                    )
                    nc.tensor.matmul(px[:, :], qh, mlos[c][:, :], start=False, stop=False)
                    nc.tensor.matmul(
                        px[:, :],
                        qlo,
                        mhis[c][:, :],
                        start=False,
                        stop=(c == DC - 1),
                    )
                maskt = work.tile([P, DV], bf16, tag="mask")
                nc.sync.dma_start(out=maskt, in_=mask_d[t * P : (t + 1) * P, :])
                xs = work.tile([P, DV], f32, tag="x")
                nc.vector.tensor_add(xs[:, :], px[:, :], maskt[:, :])
                nmx = stats.tile([P, 1], f32, tag="nmx")
                nc.vector.tensor_reduce(
                    out=nmx,
                    in_=xs[:, :],
                    axis=mybir.AxisListType.X,
                    op=mybir.AluOpType.max,
                    negate=True,
                )
                ex = work.tile([P, DV], f32, tag="e")
                sm = stats.tile([P, 1], f32, tag="sum")
                nc.scalar.activation(
                    ex[:, :],
                    xs[:, :],
                    mybir.ActivationFunctionType.Exp,
                    bias=nmx[:, :],
                    scale=1.0,
                    accum_out=sm[:, :],
                )
                rs = stats.tile([P, 1], f32, tag="r")
                nc.vector.reciprocal(rs, sm)
                ot = work.tile([P, DV], f32, tag="o")
                nc.scalar.mul(ot[:, :], ex[:, :], rs[:, :])
                nc.sync.dma_start(out=o_d[t * P : (t + 1) * P, :], in_=ot[:, :])

    nc.compile()
    return nc


def _split16_pack(x):
    """[..., n] fp32 -> [..., 2n] fp16 packed [hi | lo] along the last axis."""
    hi = x.astype(np.float16)
    lo = (x - hi.astype(np.float32)).astype(np.float16)
    return np.ascontiguousarray(np.concatenate([hi, lo], axis=-1))


def _get_nc(mm_mode: str):
    if mm_mode not in _CACHE:
        _install_ntff_hook()
        _CACHE[mm_mode] = _build(mm_mode)
    return _CACHE[mm_mode]


def kernel(K, V, Q, valid_len, _trace=False, _mm_mode=None):
    mm_mode = _mm_mode or MM_MODE
    from concourse.bass_utils import run_bass_kernel_spmd

    K = np.ascontiguousarray(np.asarray(K, dtype=np.float32))
    V = np.ascontiguousarray(np.asarray(V, dtype=np.float32))
    Q = np.asarray(Q, dtype=np.float32)
    vl = np.asarray(valid_len)

    # Q^T per batch, pre-scaled by 1/sqrt(d)
    scale = np.float32(1.0 / math.sqrt(D))
    qt = np.ascontiguousarray((Q * scale).transpose(0, 2, 1))
    # additive mask [Sq, dv] in bf16 (values are exactly 0 or ~-1e6; masked
    # lanes underflow to 0 after exp either way, identical to masked_fill)
    import ml_dtypes

    maskadd = np.where(
        np.arange(DV, dtype=np.int64)[None, :] > vl.astype(np.int64)[:, None],
        np.float32(NEG_FILL),
        np.float32(0.0),
    ).astype(ml_dtypes.bfloat16)

    nc = _get_nc(mm_mode)
    if mm_mode == "fp16x3":
        in_maps = [
            {
                "k": _split16_pack(K[b]),
                "v": _split16_pack(V[b]),
                "qt": _split16_pack(qt[b]),
                "mask": maskadd,
            }
            for b in range(N_CORES)
        ]
    else:
        in_maps = [
            {"k": K[b], "v": V[b], "qt": qt[b], "mask": maskadd}
            for b in range(N_CORES)
        ]
    res = run_bass_kernel_spmd(
        nc, in_maps, core_ids=list(range(N_CORES)), trace=_trace
    )
    out = np.stack([res.results[b]["o"] for b in range(N_CORES)], axis=0)
    if _trace:
        kernel.last_result = res
    return out


# revision 13
# speedup vs baseline: 2.2165x; 1.1875x over previous
"""Trainium2 Bass kernel for nn_DotAtt_40097814675537.

Math (matches the reference exactly up to fp rounding):
    score = Q @ K^T / sqrt(d)        [B, Sq, Sk]
    x     = score @ V                [B, Sq, dv]
    out   = softmax(where(j > valid_len[q], -1e6, x[b, q, j]), axis=-1)

Optimizations:
  * Associativity: x = (Q / sqrt(d)) @ (K^T @ V) - 4x fewer FLOPs
    (contraction 2048 -> 512 for the big matmul; no nonlinearity between
    the two matmuls so this is exact math, only fp rounding differs).
  * Data-parallel over batch B=8, one batch per NeuronCore, no collectives.
  * fp32-accurate matmuls from 3 float16 passes (hi/lo split): each fp32
    operand x = hi + lo with hi=fp16(x), lo=fp16(x-hi), then
    a@b ~= ah@bh + ah@bl + al@bh (al@bl ~ 2^-22 rel, dropped).  Runs at
    the fp16 PE rate: 3 cyc/row total vs 4 cyc/row for native fp32.
  * Sorted-query specialization: rows whose mask kills column j produce
    EXACTLY 0 in the output (exp underflows), so for each 128-row tile only
    columns [0, max(valid_len)+1) need computing.  The host sorts queries
    by valid_len (softmax is row-wise, so a row permutation is exact) and
    the kernel computes a per-tile column width; unwritten output stays 0
    (output buffers are pre-zeroed).  Host inverse-permutes the result.
    The build is cached per width-tuple, so any input data is handled
    correctly (seed-dependent widths just trigger a rebuild).
"""

import math
import sys
import types

import numpy as np

B, SQ, SK, D, DV = 8, 2048, 2048, 512, 512
N_CORES = 8
P = 128  # partitions
SC = SK // P  # 16 s-chunks for the K^T V contraction
DC = D // P  # 4 d-chunks for the Q M contraction
QT_TILES = SQ // P  # 16 query row tiles
NEG_FILL = -1000000.0

_CACHE = {}


def _install_ntff_hook():
    """antenv.axon_hooks is absent in this image; provide it so trace=True
    profiling works when requested (used by test.py, harmless otherwise)."""
    if "antenv.axon_hooks" in sys.modules:
        return
    try:
        from trn_agent_boot.trn_boot import _ntff_profile_via_ctypes

        hook = _ntff_profile_via_ctypes("/opt/axon/libaxon_pjrt.so")
    except Exception:
        hook = None
    mod = types.ModuleType("antenv.axon_hooks")
    mod.get_axon_ntff_profile_hook = lambda: hook
    mod.set_axon_ntff_profile_hook = lambda h: None
    sys.modules["antenv.axon_hooks"] = mod


def _build(widths):
    import concourse.tile as tile
    from concourse import bacc, mybir

    nc = bacc.Bacc("TRN2", target_bir_lowering=False, debug=False, num_devices=N_CORES)
    f32 = mybir.dt.float32
    f16 = mybir.dt.float16
    bf16 = mybir.dt.bfloat16

    sum_w = sum(widths)
    offs = [0]
    for w in widths:
        offs.append(offs[-1] + w)

    k_d = nc.dram_tensor("k", [SK, 2 * D], f16, kind="ExternalInput")  # [hi|lo]
    v_d = nc.dram_tensor("v", [SK, 2 * DV], f16, kind="ExternalInput")
    qt_d = nc.dram_tensor("qt", [D, 2 * SQ], f16, kind="ExternalInput")  # scaled
    mask_d = nc.dram_tensor("mask", [P, sum_w], bf16, kind="ExternalInput")
    o_d = nc.dram_tensor("o", [SQ, DV], f32, kind="ExternalOutput")

    with tile.TileContext(nc) as tc:
        with (
            tc.tile_pool(name="consts", bufs=1) as consts,
            tc.tile_pool(name="qt", bufs=1) as qt_pool,
            tc.tile_pool(name="kv", bufs=3) as kv_pool,
            tc.tile_pool(name="mprime", bufs=1) as mp_pool,
            tc.tile_pool(name="psm", bufs=1, space="PSUM") as psum_m,
            tc.tile_pool(name="psx", bufs=3, space="PSUM") as psum_x,
            tc.tile_pool(name="work", bufs=3) as work,
            tc.tile_pool(name="stats", bufs=4) as stats,
        ):
            # whole (sorted, per-tile-trimmed) additive mask in one DMA
            mask_t = consts.tile([P, sum_w], bf16, tag="mask")
            nc.scalar.dma_start(out=mask_t, in_=mask_d[:, :])

            # resident packed Q^T: 4 chunks [128, 2*2048]
            qts = []
            for c in range(DC):
                t = qt_pool.tile([P, 2 * SQ], f16, tag=f"qt{c}", name=f"qt_sb{c}")
                nc.gpsimd.dma_start(out=t, in_=qt_d[c * P : (c + 1) * P, :])
                qts.append(t)

            # Phase 1: M = K^T V over 16 s-chunks, 3 fp16 passes each
            psums = [
                psum_m.tile([P, DV], f32, tag=f"m{c}", name=f"psum_m{c}")
                for c in range(DC)
            ]
            for s in range(SC):
                kt = kv_pool.tile([P, 2 * D], f16, tag="k")
                nc.sync.dma_start(out=kt, in_=k_d[s * P : (s + 1) * P, :])
                vt = kv_pool.tile([P, 2 * DV], f16, tag="v")
                nc.gpsimd.dma_start(out=vt, in_=v_d[s * P : (s + 1) * P, :])
                vh = vt[:, 0:DV]
                vlo = vt[:, DV : 2 * DV]
                for c in range(DC):
                    kh = kt[:, c * P : (c + 1) * P]
                    klo = kt[:, D + c * P : D + (c + 1) * P]
                    # same-weight passes adjacent to reuse loaded weights
                    nc.tensor.matmul(
                        psums[c][:, :], kh, vh, start=(s == 0), stop=False
                    )
                    nc.tensor.matmul(psums[c][:, :], kh, vlo, start=False, stop=False)
                    nc.tensor.matmul(
                        psums[c][:, :], klo, vh, start=False, stop=(s == SC - 1)
                    )

            # M PSUM -> SBUF split into fp16 hi/lo (ScalarE cast + DVE residual)
            mhis, mlos = [], []
            for c in range(DC):
                mhi = mp_pool.tile([P, DV], f16, tag=f"mh{c}", name=f"mhi{c}")
                nc.scalar.copy(mhi[:, :], psums[c][:, :])
                mlo = mp_pool.tile([P, DV], f16, tag=f"ml{c}", name=f"mlo{c}")
                nc.vector.tensor_sub(mlo[:, :], psums[c][:, :], mhi[:, :])
                mhis.append(mhi)
                mlos.append(mlo)

            # Phase 2: per query tile (width W): X = Q M, mask, softmax, store
            for t in range(QT_TILES):
                W = widths[t]
                px = psum_x.tile([P, DV], f32, tag="x")
                for c in range(DC):
                    qh = qts[c][:, t * P : (t + 1) * P]
                    qlo = qts[c][:, SQ + t * P : SQ + (t + 1) * P]
                    nc.tensor.matmul(
                        px[:, 0:W], qh, mhis[c][:, 0:W], start=(c == 0), stop=False
                    )
                    nc.tensor.matmul(
                        px[:, 0:W], qh, mlos[c][:, 0:W], start=False, stop=False
                    )
                    nc.tensor.matmul(
                        px[:, 0:W],
                        qlo,
                        mhis[c][:, 0:W],
                        start=False,
                        stop=(c == DC - 1),
                    )
                xs = work.tile([P, DV], f32, tag="x")
                nc.vector.tensor_add(
                    xs[:, 0:W], px[:, 0:W], mask_t[:, offs[t] : offs[t] + W]
                )
                nmx = stats.tile([P, 1], f32, tag="nmx")
                nc.vector.tensor_reduce(
                    out=nmx,
                    in_=xs[:, 0:W],
                    axis=mybir.AxisListType.X,
                    op=mybir.AluOpType.max,
                    negate=True,
                )
                ex = work.tile([P, DV], f32, tag="e")
                sm = stats.tile([P, 1], f32, tag="sum")
                nc.scalar.activation(
                    ex[:, 0:W],
                    xs[:, 0:W],
                    mybir.ActivationFunctionType.Exp,
                    bias=nmx[:, :],
                    scale=1.0,
                    accum_out=sm[:, :],
                )
                rs = stats.tile([P, 1], f32, tag="r")
                nc.vector.reciprocal(rs, sm)
                ot = work.tile([P, DV], f32, tag="o")
                nc.scalar.mul(ot[:, 0:W], ex[:, 0:W], rs[:, :])
                nc.scalar.dma_start(
                    out=o_d[t * P : (t + 1) * P, 0:W], in_=ot[:, 0:W]
                )

    nc.compile()
    return nc


def _split16_pack(x):
    """[..., n] fp32 -> [..., 2n] fp16 packed [hi | lo] along the last axis."""
    hi = x.astype(np.float16)
    lo = (x - hi.astype(np.float32)).astype(np.float16)
    return np.ascontiguousarray(np.concatenate([hi, lo], axis=-1))


def _get_nc(widths):
    key = tuple(widths)
    if key not in _CACHE:
        _install_ntff_hook()
        _CACHE[key] = _build(key)
    return _CACHE[key]


def kernel(K, V, Q, valid_len, _trace=False):
    import ml_dtypes

    from concourse.bass_utils import run_bass_kernel_spmd

    K = np.ascontiguousarray(np.asarray(K, dtype=np.float32))
    V = np.ascontiguousarray(np.asarray(V, dtype=np.float32))
    Q = np.asarray(Q, dtype=np.float32)
    vl = np.asarray(valid_len).astype(np.int64)

    # sort queries by valid_len (row permutation; exact for row-wise softmax)
    perm = np.argsort(vl, kind="stable")
    vls = vl[perm]
    widths = []
    for t in range(QT_TILES):
        w = int(vls[t * P : (t + 1) * P].max()) + 1
        widths.append(min(DV, -(-w // 32) * 32))
    widths = tuple(widths)
    sum_w = sum(widths)

    # Q^T per batch: permuted rows, pre-scaled by 1/sqrt(d), fp16 hi/lo packed
    scale = np.float32(1.0 / math.sqrt(D))
    qp = Q[:, perm, :] * scale
    qt = np.ascontiguousarray(qp.transpose(0, 2, 1))

    # additive mask for the sorted rows, packed per tile: [128, sum_w] bf16
    # (bf16 is exact here: values are only 0 / -1e6-ish; masked lanes
    # underflow to 0 after exp either way, identical to masked_fill)
    col = np.arange(DV, dtype=np.int64)
    mask_full = np.where(
        col[None, :] > vls[:, None], np.float32(NEG_FILL), np.float32(0.0)
    )
    mask_packed = np.empty((P, sum_w), dtype=ml_dtypes.bfloat16)
    off = 0
    for t in range(QT_TILES):
        w = widths[t]
        mask_packed[:, off : off + w] = mask_full[t * P : (t + 1) * P, :w].astype(
            ml_dtypes.bfloat16
        )
        off += w

    nc = _get_nc(widths)
    in_maps = [
        {
            "k": _split16_pack(K[b]),
            "v": _split16_pack(V[b]),
            "qt": _split16_pack(qt[b]),
            "mask": mask_packed,
        }
        for b in range(N_CORES)
    ]
    res = run_bass_kernel_spmd(
        nc, in_maps, core_ids=list(range(N_CORES)), trace=_trace
    )
    # device rows r correspond to original queries perm[r]; unwritten
    # (masked) columns stay 0 from the pre-zeroed output buffers
    out = np.empty((B, SQ, DV), dtype=np.float32)
    for b in range(N_CORES):
        out[b, perm, :] = res.results[b]["o"]
    if _trace:
        kernel.last_result = res
    return out


# revision 16
# speedup vs baseline: 2.3515x; 1.0609x over previous
"""Trainium2 Bass kernel for nn_DotAtt_40097814675537.

Math (matches the reference exactly up to fp rounding):
    score = Q @ K^T / sqrt(d)        [B, Sq, Sk]
    x     = score @ V                [B, Sq, dv]
    out   = softmax(where(j > valid_len[q], -1e6, x[b, q, j]), axis=-1)

Optimizations:
  * Associativity: x = (Q / sqrt(d)) @ (K^T @ V) - 4x fewer FLOPs
    (contraction 2048 -> 512 for the big matmul; no nonlinearity between
    the two matmuls so this is exact math, only fp rounding differs).
  * Data-parallel over batch B=8, one batch per NeuronCore, no collectives.
  * fp32-accurate matmuls from 3 float16 passes (hi/lo split): each fp32
    operand x = hi + lo with hi=fp16(x), lo=fp16(x-hi), then
    a@b ~= ah@bh + ah@bl + al@bh (al@bl ~ 2^-22 rel, dropped).  Runs at
    the fp16 PE rate: 3 cyc/row total vs 4 cyc/row for native fp32.
  * Sorted-query specialization: rows whose mask kills column j produce
    EXACTLY 0 in the output (exp underflows), so for each 128-row tile only
    columns [0, max(valid_len)+1) need computing.  The host sorts queries
    by valid_len (softmax is row-wise, so a row permutation is exact) and
    the kernel computes a per-tile column width; unwritten output stays 0
    (output buffers are pre-zeroed).  Host inverse-permutes the result.
    The build is cached per width-tuple, so any input data is handled
    correctly (seed-dependent widths just trigger a rebuild).
"""

import math
import sys
import types

import numpy as np

B, SQ, SK, D, DV = 8, 2048, 2048, 512, 512
N_CORES = 8
P = 128  # partitions
SC = SK // P  # 16 s-chunks for the K^T V contraction
DC = D // P  # 4 d-chunks for the Q M contraction
QT_TILES = SQ // P  # 16 query row tiles
NEG_FILL = -1000000.0

_CACHE = {}


def _install_ntff_hook():
    """antenv.axon_hooks is absent in this image; provide it so trace=True
    profiling works when requested (used by test.py, harmless otherwise)."""
    if "antenv.axon_hooks" in sys.modules:
        return
    try:
        from trn_agent_boot.trn_boot import _ntff_profile_via_ctypes

        hook = _ntff_profile_via_ctypes("/opt/axon/libaxon_pjrt.so")
    except Exception:
        hook = None
    mod = types.ModuleType("antenv.axon_hooks")
    mod.get_axon_ntff_profile_hook = lambda: hook
    mod.set_axon_ntff_profile_hook = lambda h: None
    sys.modules["antenv.axon_hooks"] = mod


def _build(widths):
    import concourse.tile as tile
    from concourse import bacc, mybir

    nc = bacc.Bacc("TRN2", target_bir_lowering=False, debug=False, num_devices=N_CORES)
    f32 = mybir.dt.float32
    f16 = mybir.dt.float16
    bf16 = mybir.dt.bfloat16

    sum_w = sum(widths)
    offs = [0]
    for w in widths:
        offs.append(offs[-1] + w)

    k_d = nc.dram_tensor("k", [SK, 2 * D], f16, kind="ExternalInput")  # [hi|lo]
    v_d = nc.dram_tensor("v", [SK, 2 * DV], f16, kind="ExternalInput")
    qt_d = nc.dram_tensor("qt", [D, 2 * SQ], f16, kind="ExternalInput")  # scaled
    mask_d = nc.dram_tensor("mask", [P, sum_w], bf16, kind="ExternalInput")
    o_d = nc.dram_tensor("o", [SQ, DV], f32, kind="ExternalOutput")

    with tile.TileContext(nc) as tc:
        with (
            tc.tile_pool(name="consts", bufs=1) as consts,
            tc.tile_pool(name="qt", bufs=1) as qt_pool,
            tc.tile_pool(name="kv", bufs=4) as kv_pool,
            tc.tile_pool(name="mprime", bufs=1) as mp_pool,
            tc.tile_pool(name="psm", bufs=1, space="PSUM") as psum_m,
            tc.tile_pool(name="psx", bufs=3, space="PSUM") as psum_x,
            tc.tile_pool(name="work", bufs=3) as work,
            tc.tile_pool(name="stats", bufs=4) as stats,
        ):
            # whole (sorted, per-tile-trimmed) additive mask in one DMA
            mask_t = consts.tile([P, sum_w], bf16, tag="mask")
            nc.scalar.dma_start(out=mask_t, in_=mask_d[:, :])

            # Phase 1: M = K^T V over 16 s-chunks, 3 fp16 passes each
            psums = [
                psum_m.tile([P, DV], f32, tag=f"m{c}", name=f"psum_m{c}")
                for c in range(DC)
            ]
            for s in range(SC):
                kt = kv_pool.tile([P, 2 * D], f16, tag="k")
                nc.sync.dma_start(out=kt, in_=k_d[s * P : (s + 1) * P, :])
                vt = kv_pool.tile([P, 2 * DV], f16, tag="v")
                nc.gpsimd.dma_start(out=vt, in_=v_d[s * P : (s + 1) * P, :])
                vh = vt[:, 0:DV]
                vlo = vt[:, DV : 2 * DV]
                for c in range(DC):
                    kh = kt[:, c * P : (c + 1) * P]
                    klo = kt[:, D + c * P : D + (c + 1) * P]
                    # same-weight passes adjacent to reuse loaded weights
                    nc.tensor.matmul(
                        psums[c][:, :], kh, vh, start=(s == 0), stop=False
                    )
                    nc.tensor.matmul(psums[c][:, :], kh, vlo, start=False, stop=False)
                    nc.tensor.matmul(
                        psums[c][:, :], klo, vh, start=False, stop=(s == SC - 1)
                    )

            # resident packed Q^T: 4 chunks [128, 2*2048].  Issued AFTER the
            # K/V stream so the phase-1-critical chunks win the DMA queues;
            # QT data is not needed until phase 2 (~45us in).
            qts = []
            for c in range(DC):
                t = qt_pool.tile([P, 2 * SQ], f16, tag=f"qt{c}", name=f"qt_sb{c}")
                nc.gpsimd.dma_start(out=t, in_=qt_d[c * P : (c + 1) * P, :])
                qts.append(t)

            # M PSUM -> SBUF split into fp16 hi/lo (ScalarE cast + DVE residual)
            mhis, mlos = [], []
            for c in range(DC):
                mhi = mp_pool.tile([P, DV], f16, tag=f"mh{c}", name=f"mhi{c}")
                nc.scalar.copy(mhi[:, :], psums[c][:, :])
                mlo = mp_pool.tile([P, DV], f16, tag=f"ml{c}", name=f"mlo{c}")
                nc.vector.tensor_sub(mlo[:, :], psums[c][:, :], mhi[:, :])
                mhis.append(mhi)
                mlos.append(mlo)

            # Phase 2: per query tile (width W): X = Q M, mask, softmax, store
            for t in range(QT_TILES):
                W = widths[t]
                px = psum_x.tile([P, DV], f32, tag="x")
                for c in range(DC):
                    qh = qts[c][:, t * P : (t + 1) * P]
                    qlo = qts[c][:, SQ + t * P : SQ + (t + 1) * P]
                    nc.tensor.matmul(
                        px[:, 0:W], qh, mhis[c][:, 0:W], start=(c == 0), stop=False
                    )
                    nc.tensor.matmul(
                        px[:, 0:W], qh, mlos[c][:, 0:W], start=False, stop=False
                    )
                    nc.tensor.matmul(
                        px[:, 0:W],
                        qlo,
                        mhis[c][:, 0:W],
                        start=False,
                        stop=(c == DC - 1),
                    )
                xs = work.tile([P, DV], f32, tag="x")
                nc.vector.tensor_add(
                    xs[:, 0:W], px[:, 0:W], mask_t[:, offs[t] : offs[t] + W]
                )
                nmx = stats.tile([P, 1], f32, tag="nmx")
                nc.vector.tensor_reduce(
                    out=nmx,
                    in_=xs[:, 0:W],
                    axis=mybir.AxisListType.X,
                    op=mybir.AluOpType.max,
                    negate=True,
                )
                ex = work.tile([P, DV], f32, tag="e")
                sm = stats.tile([P, 1], f32, tag="sum")
                nc.scalar.activation(
                    ex[:, 0:W],
                    xs[:, 0:W],
                    mybir.ActivationFunctionType.Exp,
                    bias=nmx[:, :],
                    scale=1.0,
                    accum_out=sm[:, :],
                )
                rs = stats.tile([P, 1], f32, tag="r")
                nc.vector.reciprocal(rs, sm)
                ot = work.tile([P, DV], f32, tag="o")
                nc.scalar.mul(ot[:, 0:W], ex[:, 0:W], rs[:, :])
                nc.scalar.dma_start(
                    out=o_d[t * P : (t + 1) * P, 0:W], in_=ot[:, 0:W]
                )

    nc.compile()
    return nc


def _split16_pack(x):
    """[..., n] fp32 -> [..., 2n] fp16 packed [hi | lo] along the last axis."""
    hi = x.astype(np.float16)
    lo = (x - hi.astype(np.float32)).astype(np.float16)
    return np.ascontiguousarray(np.concatenate([hi, lo], axis=-1))


def _get_nc(widths):
    key = tuple(widths)
    if key not in _CACHE:
        _install_ntff_hook()
        _CACHE[key] = _build(key)
    return _CACHE[key]


def kernel(K, V, Q, valid_len, _trace=False):
    import ml_dtypes

    from concourse.bass_utils import run_bass_kernel_spmd

    K = np.ascontiguousarray(np.asarray(K, dtype=np.float32))
    V = np.ascontiguousarray(np.asarray(V, dtype=np.float32))
    Q = np.asarray(Q, dtype=np.float32)
    vl = np.asarray(valid_len).astype(np.int64)

    # sort queries by valid_len (row permutation; exact for row-wise softmax)
    perm = np.argsort(vl, kind="stable")
    vls = vl[perm]
    widths = []
    for t in range(QT_TILES):
        w = int(vls[t * P : (t + 1) * P].max()) + 1
        widths.append(min(DV, -(-w // 32) * 32))
    widths = tuple(widths)
    sum_w = sum(widths)

    # Q^T per batch: permuted rows, pre-scaled by 1/sqrt(d), fp16 hi/lo packed
    scale = np.float32(1.0 / math.sqrt(D))
    qp = Q[:, perm, :] * scale
    qt = np.ascontiguousarray(qp.transpose(0, 2, 1))

    # additive mask for the sorted rows, packed per tile: [128, sum_w] bf16
    # (bf16 is exact here: values are only 0 / -1e6-ish; masked lanes
    # underflow to 0 after exp either way, identical to masked_fill)
    col = np.arange(DV, dtype=np.int64)
    mask_full = np.where(
        col[None, :] > vls[:, None], np.float32(NEG_FILL), np.float32(0.0)
    )
    mask_packed = np.empty((P, sum_w), dtype=ml_dtypes.bfloat16)
    off = 0
    for t in range(QT_TILES):
        w = widths[t]
        mask_packed[:, off : off + w] = mask_full[t * P : (t + 1) * P, :w].astype(
            ml_dtypes.bfloat16
        )
        off += w

    nc = _get_nc(widths)
    in_maps = [
        {
            "k": _split16_pack(K[b]),
            "v": _split16_pack(V[b]),
            "qt": _split16_pack(qt[b]),
            "mask": mask_packed,
        }
        for b in range(N_CORES)
    ]
    res = run_bass_kernel_spmd(
        nc, in_maps, core_ids=list(range(N_CORES)), trace=_trace
    )
    # device rows r correspond to original queries perm[r]; unwritten
    # (masked) columns stay 0 from the pre-zeroed output buffers
    out = np.empty((B, SQ, DV), dtype=np.float32)
    for b in range(N_CORES):
        out[b, perm, :] = res.results[b]["o"]
    if _trace:
        kernel.last_result = res
    return out
